# revision 1
# baseline (speedup 1.0000x reference)
"""DiffusionGraphConv Trainium2 kernel (8-core SPMD, data-parallel over batch).

Math (per reference):
  x = concat(inputs, state)           -> [B, N, F]   B=32, N=4096, F=128
  x0 = x transposed to [N, F*B]
  per support s (2): x1 = A_s x0 ; x2 = 2 A_s x1 - x0   (A_s dense from COO)
  out[b*N+n, o] = sum_{f,m} xs_m[n, f, b] * W[f*M+m, o] + bias[o]

Sharding: batch across 8 cores (4 batches/core, C = 4*F = 512 columns of x0).

Device algorithm per core (all matmuls on TensorE):
  Phase 1 (per support): X1s = 2 * (A_s @ X0)  via fp32r matmuls
     (lhsT = AT tiles [128n_in, 128n_out], rhs = X0 tiles [128n_in, 512c]),
     evacuated to bf16.
  Phase 2 (per 512-wide output-node chunk rc, per local batch j):
     x0T chunk  : PE transpose of X0 tiles (fp32r)
     x1T chunks : PE transpose of X1s tiles (bf16)
     x2T chunk  : flipped application (A_s @ X1s)^T = lhsT=X1s tile,
                  rhs = AT[:, chunk] (bf16 matmuls), minus x0T (DVE)
     out^T[o, chunk] = sum_m V_m^T @ xmT_chunk   (5 fp32r matmuls, PSUM acc)
     (+ bias via ACT on evacuation)
  V_m = W[:, m, :] with the Chebyshev "2x" folded: V for x1 terms = W/2.

Everything is hardcoded for the reference shapes; host does only layout prep
(dense-ify supports, transpose/shard x0, tile A) and output reassembly.
"""

import numpy as np
import ml_dtypes

import concourse.bass as bass
import concourse.tile as tile
from concourse import bacc, mybir
from concourse import bass_utils

B, N, D, H, O, S = 32, 4096, 64, 64, 128, 2
F = D + H                    # 128
NCORES = 8
BLOC = B // NCORES           # 4 batches per core
C = BLOC * F                 # 512 columns per core
NBLK = N // 128              # 32 n-tiles
NRC = N // 512               # 8 output-node chunks
M = 5

F32 = mybir.dt.float32
F32R = mybir.dt.float32r
BF16 = mybir.dt.bfloat16

_CACHE = {}


def build_nc():
    nc = bacc.Bacc("TRN2", target_bir_lowering=False, debug=False)

    # ---- DRAM tensors ----
    # x0 natural n-tiles: x0[t][p, c] = x0_core[t*128+p, c], c = j*128+f
    x0_d = nc.dram_tensor("x0", [NBLK, 128, C], F32R, kind="ExternalInput")
    # A for phase 1: a1[s, i, p, k, q] = AT_s[k*128+p, i*128+q]  (fp32)
    a1_d = nc.dram_tensor("a1", [S, NBLK, 128, NBLK, 128], F32R,
                          kind="ExternalInput")
    # A for phase 2: a2[s, rc, k, p, q] = AT_s[k*128+p, rc*512+q]  (bf16)
    a2_d = nc.dram_tensor("a2", [S, NRC, NBLK, 128, 512], BF16,
                          kind="ExternalInput")
    # weights V[m][f, o]; bias [o, 1]; identities
    v_d = nc.dram_tensor("v", [M, 128, 128], F32R, kind="ExternalInput")
    bias_d = nc.dram_tensor("bias", [128, 1], F32, kind="ExternalInput")
    idn_d = nc.dram_tensor("idn", [128, 128], F32R, kind="ExternalInput")
    idnb_d = nc.dram_tensor("idnb", [128, 128], BF16, kind="ExternalInput")
    # output: out[j, o, n]
    out_d = nc.dram_tensor("out", [BLOC, 128, N], F32, kind="ExternalOutput")

    A1Q = 8      # k-tiles per a1 quarter-slab DMA
    NQ = NBLK // A1Q

    with tile.TileContext(nc) as tc:
        with (
            tc.tile_pool(name="big", bufs=1) as big,
            tc.tile_pool(name="a1p", bufs=6) as a1p,
            tc.tile_pool(name="a2p", bufs=8) as a2p,
            tc.tile_pool(name="stg", bufs=1) as stg,
            tc.tile_pool(name="ps2", bufs=2, space=bass.MemorySpace.PSUM) as ps2,
            tc.tile_pool(name="ps1", bufs=1, space=bass.MemorySpace.PSUM) as ps1,
        ):
            # ---- load resident tensors ----
            NXQ = NBLK // 4
            x0q = [big.tile([128, NXQ, C], F32R, tag=f"x0q{q}",
                            name=f"x0q{q}") for q in range(4)]
            def x0k(t):
                return x0q[t // NXQ][:, t % NXQ, :]

            def fetch_a1(s, i):
                qs = []
                for h in range(NQ):
                    aq = a1p.tile([128, A1Q, 128], F32R, tag="a1",
                                  name=f"a1_{s}_{i}_{h}")
                    nc.sync.dma_start(
                        aq[:], a1_d[s, i, :, h * A1Q:(h + 1) * A1Q, :])
                    qs.append(aq)
                return qs

            # interleave first A slab with x0 load so PE starts early
            for t in range(8):
                nc.sync.dma_start(x0k(t), x0_d[t])
            a1_pre = {(0, 0): fetch_a1(0, 0)}
            for t in range(8, 16):
                nc.sync.dma_start(x0k(t), x0_d[t])
            a1_pre[(0, 1)] = fetch_a1(0, 1)
            for t in range(16, NBLK):
                nc.sync.dma_start(x0k(t), x0_d[t])
            vsb = big.tile([128, M, 128], F32R, tag="v")
            for m in range(M):
                nc.sync.dma_start(vsb[:, m, :], v_d[m])
            bias_sb = big.tile([128, 1], F32, tag="bias")
            nc.sync.dma_start(bias_sb[:], bias_d[:])
            idn = big.tile([128, 128], F32R, tag="idn")
            nc.sync.dma_start(idn[:], idn_d[:])
            idnb = big.tile([128, 128], BF16, tag="idnb")
            nc.sync.dma_start(idnb[:], idnb_d[:])

            # ---- phase 1: X1s[s] = 2 * A_s @ X0  (bf16 out) ----
            x1sb = []
            for s in range(S):
                x1 = big.tile([128, NBLK, C], BF16, tag=f"x1_{s}")
                x1sb.append(x1)
                for i in range(NBLK):
                    quarters = a1_pre.pop((s, i), None) or fetch_a1(s, i)
                    ps = ps2.tile([128, C], F32, tag="acc")
                    for k in range(NBLK):
                        nc.tensor.matmul(
                            ps[:],
                            quarters[k // A1Q][:, k % A1Q, :],
                            x0k(k),
                            start=(k == 0),
                            stop=(k == NBLK - 1),
                        )
                    nc.scalar.mul(x1[:, i, :], ps[:], 2.0)

            # ---- phase 2: per output-node chunk rc ----
            for rc in range(NRC):
                # x0T chunks for each j (used by m0 term and both subtracts)
                x0t = []
                for j in range(BLOC):
                    ptr = ps1.tile([128, 512], F32R, tag="trx", bufs=2)
                    for t in range(4):
                        nt = rc * 4 + t
                        nc.tensor.transpose(
                            ptr[:, t * 128:(t + 1) * 128],
                            x0k(nt)[:, j * 128:(j + 1) * 128],
                            idn[:])
                    st = stg.tile([128, 512], F32R, tag="x0t", bufs=6)
                    nc.scalar.copy(st[:], ptr[:])
                    x0t.append(st)

                # x2T chunks: flipped application, then subtract x0T
                x2t = {}
                for s in range(S):
                    pxj = [ps1.tile([128, 512], F32, tag=f"px2_{j}",
                                    name=f"px2_{s}_{rc}_{j}")
                           for j in range(BLOC)]
                    for k in range(NBLK):
                        a2t = a2p.tile([128, 512], BF16, tag="a2")
                        nc.sync.dma_start(a2t[:], a2_d[s, rc, k])
                        for j in range(BLOC):
                            nc.tensor.matmul(
                                pxj[j][:],
                                x1sb[s][:, k, j * 128:(j + 1) * 128],
                                a2t[:],
                                start=(k == 0),
                                stop=(k == NBLK - 1),
                            )
                    for j in range(BLOC):
                        st = stg.tile([128, 512], F32R, tag="x2t", bufs=8)
                        nc.vector.tensor_sub(st[:], pxj[j][:], x0t[j][:])
                        x2t[(s, j)] = st

                # out^T accumulation per j
                for j in range(BLOC):
                    x1t = {}
                    for s in range(S):
                        ptb = ps1.tile([128, 512], BF16, tag="trx", bufs=2)
                        for t in range(4):
                            nt = rc * 4 + t
                            nc.tensor.transpose(
                                ptb[:, t * 128:(t + 1) * 128],
                                x1sb[s][:, nt, j * 128:(j + 1) * 128],
                                idnb[:])
                        st = stg.tile([128, 512], F32R, tag="x1t", bufs=4)
                        nc.scalar.copy(st[:], ptb[:])
                        x1t[s] = st

                    po = ps2.tile([128, 512], F32, tag="acc")
                    rhs_seq = [
                        (0, x0t[j]),
                        (1, x1t[0]),
                        (2, x2t[(0, j)]),
                        (3, x1t[1]),
                        (4, x2t[(1, j)]),
                    ]
                    for idx, (m, st) in enumerate(rhs_seq):
                        nc.tensor.matmul(
                            po[:],
                            vsb[:, m, :],
                            st[:],
                            start=(idx == 0),
                            stop=(idx == len(rhs_seq) - 1),
                        )
                    ot = stg.tile([128, 512], F32, tag="ot", bufs=4)
                    if j % 2 == 0:
                        nc.scalar.add(ot[:], po[:], bias_sb[:, 0:1])
                    else:
                        nc.vector.tensor_scalar_add(ot[:], po[:], bias_sb[:, 0:1])
                    nc.sync.dma_start(
                        out_d[j, :, rc * 512:(rc + 1) * 512], ot[:])

    nc.compile()
    return nc


def _prep_shared(sup_rows, sup_cols, sup_vals, weight, biases):
    AT = np.zeros((S, N, N), dtype=np.float32)
    for s in range(S):
        np.add.at(AT[s], (sup_cols[s].astype(np.int64),
                          sup_rows[s].astype(np.int64)),
                  sup_vals[s].astype(np.float32))
    # a1[s, i, p, k, q] = AT[s][k*128+p, i*128+q]
    a1 = np.ascontiguousarray(
        AT.reshape(S, NBLK, 128, NBLK, 128).transpose(0, 3, 2, 1, 4))
    # a2[s, rc, k, p, q] = AT[s][k*128+p, rc*512+q]
    a2 = np.ascontiguousarray(
        AT.reshape(S, NBLK, 128, NRC, 512).transpose(0, 3, 1, 2, 4)
    ).astype(ml_dtypes.bfloat16)

    Wm = np.asarray(weight, dtype=np.float32).reshape(F, M, O)
    V = np.stack([Wm[:, 0, :], Wm[:, 1, :] * 0.5, Wm[:, 2, :],
                  Wm[:, 3, :] * 0.5, Wm[:, 4, :]]).astype(np.float32)
    V = np.ascontiguousarray(V)
    bias = np.asarray(biases, dtype=np.float32).reshape(128, 1)
    idn = np.eye(128, dtype=np.float32)
    idnb = np.eye(128, dtype=ml_dtypes.bfloat16)
    return a1, a2, V, bias, idn, idnb


def kernel(inputs, state, sup_rows, sup_cols, sup_vals, weight, biases,
           output_size=128, **_ignored):
    inputs = np.asarray(inputs, dtype=np.float32)
    state = np.asarray(state, dtype=np.float32)
    x = np.concatenate(
        [inputs.reshape(B, N, D), state.reshape(B, N, H)], axis=2)  # [B,N,F]

    a1, a2, V, bias, idn, idnb = _prep_shared(
        np.asarray(sup_rows), np.asarray(sup_cols), np.asarray(sup_vals),
        weight, biases)

    if "nc" not in _CACHE:
        _CACHE["nc"] = build_nc()
    nc = _CACHE["nc"]

    in_maps = []
    for c in range(NCORES):
        xc = np.ascontiguousarray(
            x[c * BLOC:(c + 1) * BLOC].transpose(1, 0, 2).reshape(
                NBLK, 128, C))
        in_maps.append({
            "x0": xc, "a1": a1, "a2": a2, "v": V, "bias": bias,
            "idn": idn, "idnb": idnb,
        })

    res = None
    for attempt in range(3):
        try:
            res = bass_utils.run_bass_kernel_spmd(
                nc, in_maps, core_ids=list(range(NCORES)), trace=False)
            break
        except Exception:
            if attempt == 2:
                raise
            import time as _time
            _time.sleep(15 * (attempt + 1))

    # reassemble: out_core[j, o, n] -> out[b, n, o]
    outs = np.stack([res.results[c]["out"] for c in range(NCORES)])
    full = outs.transpose(0, 1, 3, 2).reshape(B, N, O)
    return np.ascontiguousarray(full.reshape(B, N * O))



# revision 2
# speedup vs baseline: 1.8083x; 1.8083x over previous
"""DiffusionGraphConv Trainium2 kernel (8-core SPMD, fp8-DoubleRow design).

Math (per reference, B=32, N=4096, F=128, O=128):
  x = concat(inputs, state)  -> [B, N, F];  x1 = A_s x ; x2 = (2A_s^2 - I) x
  out = sum_m xs_m @ W_m + bias

Reassociation: with B_s = 2 A_s^2 and Y_m = x @ V_m (V = [W1 W2 W3 W4]):
  out = x (W0 - W2 - W4) + A_1 Y_1 + B_1 Y_2 + A_2 Y_3 + B_2 Y_4
No Chebyshev dependency chain: all four propagations stream their (dense,
fp8-quantized, power-of-2-scaled) matrix once through the TensorE in
DoubleRow mode (256-row contraction @ 0.5 cyc/row = 4x fp32r MAC rate).

Sharding: batch across 8 cores (4 batches/core).

Per-core device schedule:
  1. transpose x0 (bf16) -> x0T[j] = [F, N]
  2. Y-build: per (j, k-tile): Y[n-tile, 4*128] = x0T-tile^T @ [V1..V4],
     ACT-evacuated to fp8
  3. out-pass: per 512-node chunk rc, per j: one PSUM chain =
     start matmul (s*(W0-W2-W4))^T @ x0T-chunk  +  64 fp8 DoubleRow matmuls
     (Y[k-pair] stationary, s*AT_m[k-pair, chunk] moving),
     ACT evacuation out = psum * (1/s) + bias -> bf16 -> DRAM
"""

import numpy as np
import ml_dtypes

import concourse.bass as bass
import concourse.tile as tile
from concourse import bacc, mybir
from concourse import bass_utils

B, N, D, H, O, S = 32, 4096, 64, 64, 128, 2
F = D + H                    # 128
NCORES = 8
BLOC = B // NCORES           # 4 batches per core
NBLK = N // 128              # 32 n-tiles
NRC = N // 512               # 8 output-node chunks
NPAIR = NBLK // 2            # 16 DoubleRow k-pairs
NM = 4                       # propagation matrices: A1, B1, A2, B2

F32 = mybir.dt.float32
BF16 = mybir.dt.bfloat16
FP8 = mybir.dt.float8e4
DRMODE = mybir.MatmulPerfMode.DoubleRow

_f8 = ml_dtypes.float8_e4m3
_bf = ml_dtypes.bfloat16

_CACHE = {}


def build_nc():
    nc = bacc.Bacc("TRN2", target_bir_lowering=False, debug=False)

    # x0 natural n-tiles: x0[t][p, c] = x0_core[t*128+p, c], c = j*128+f
    x0_d = nc.dram_tensor("x0", [NBLK, 128, BLOC * F], BF16,
                          kind="ExternalInput")
    # am[m, rc, h, p, i, q] = (s*AT_m)[(2h+i)*128+p, rc*512+q]
    am_d = nc.dram_tensor("am", [NM, NRC, NPAIR, 128, 2, 512], FP8,
                          kind="ExternalInput")
    vcat_d = nc.dram_tensor("vcat", [128, 512], BF16, kind="ExternalInput")
    v0s_d = nc.dram_tensor("v0s", [128, 128], BF16, kind="ExternalInput")
    bias_d = nc.dram_tensor("bias", [128, 1], F32, kind="ExternalInput")
    idnb_d = nc.dram_tensor("idnb", [128, 128], BF16, kind="ExternalInput")
    sc_d = nc.dram_tensor("sc", [128, 1], F32, kind="ExternalInput")
    # out[j, o, n] = out_core^T per batch
    out_d = nc.dram_tensor("out", [BLOC, 128, N], BF16, kind="ExternalOutput")

    with tile.TileContext(nc) as tc:
        with (
            tc.tile_pool(name="big", bufs=1) as big,
            tc.tile_pool(name="amp", bufs=28) as amp,
            tc.tile_pool(name="stg", bufs=1) as stg,
            tc.tile_pool(name="pst", bufs=2, space=bass.MemorySpace.PSUM) as pst,
            tc.tile_pool(name="pso", bufs=4, space=bass.MemorySpace.PSUM) as pso,
        ):
            # ---- resident loads ----
            x0 = big.tile([128, NBLK, BLOC * F], BF16, tag="x0")
            for t in range(NBLK):
                nc.sync.dma_start(x0[:, t, :], x0_d[t])
            vcat = big.tile([128, 512], BF16, tag="vcat")
            nc.sync.dma_start(vcat[:], vcat_d[:])
            v0s = big.tile([128, 128], BF16, tag="v0s")
            nc.sync.dma_start(v0s[:], v0s_d[:])
            bias_sb = big.tile([128, 1], F32, tag="bias")
            nc.sync.dma_start(bias_sb[:], bias_d[:])
            idnb = big.tile([128, 128], BF16, tag="idnb")
            nc.sync.dma_start(idnb[:], idnb_d[:])
            sc_sb = big.tile([128, 1], F32, tag="sc")
            nc.sync.dma_start(sc_sb[:], sc_d[:])

            # x0T[j] = [128 f, N], Y[j] fp8 [128 n-part, k, 4m*128o]
            x0t = big.tile([128, BLOC, N], BF16, tag="x0t")
            y = big.tile([128, BLOC, NBLK, 512], FP8, tag="y")

            # ---- phase 1: transposes + Y-build ----
            for j in range(BLOC):
                for kq in range(NBLK // 4):
                    pt = pst.tile([128, 512], BF16, tag="pt")
                    for t in range(4):
                        k = kq * 4 + t
                        nc.tensor.transpose(
                            pt[:, t * 128:(t + 1) * 128],
                            x0[:, k, j * 128:(j + 1) * 128], idnb[:])
                    nc.scalar.copy(x0t[:, j, kq * 512:(kq + 1) * 512], pt[:])
            for j in range(BLOC):
                for k in range(NBLK):
                    py = pst.tile([128, 512], F32, tag="py")
                    nc.tensor.matmul(
                        py[:], x0t[:, j, k * 128:(k + 1) * 128], vcat[:],
                        start=True, stop=True)
                    nc.scalar.copy(y[:, j, k, :], py[:])

            # ---- phase 2: out-pass ----
            for rc in range(NRC):
                po = [pso.tile([128, 512], F32, tag="po",
                               name=f"po_{rc}_{j}") for j in range(BLOC)]
                for j in range(BLOC):
                    nc.tensor.matmul(
                        po[j][:], v0s[:],
                        x0t[:, j, rc * 512:(rc + 1) * 512],
                        start=True, stop=False)
                for m in range(NM):
                    for h in range(NPAIR):
                        at = amp.tile([128, 2, 512], FP8, tag="am")
                        nc.sync.dma_start(at[:], am_d[m, rc, h])
                        last = (m == NM - 1) and (h == NPAIR - 1)
                        for j in range(BLOC):
                            nc.tensor.matmul(
                                po[j][:],
                                y[:, j, 2 * h:2 * h + 2,
                                  m * 128:(m + 1) * 128],
                                at[:],
                                start=False, stop=last,
                                perf_mode=DRMODE)
                for j in range(BLOC):
                    ot = stg.tile([128, 512], BF16, tag="ot", bufs=8)
                    nc.scalar.activation(
                        ot[:], po[j][:],
                        mybir.ActivationFunctionType.Identity,
                        bias=bias_sb[:, 0:1], scale=sc_sb[:, 0:1])
                    nc.sync.dma_start(
                        out_d[j, :, rc * 512:(rc + 1) * 512], ot[:])

    nc.compile()
    return nc


def _dense_at(sup_rows, sup_cols, sup_vals):
    """AT_s dense [S, N, N]: AT[c, r] = sum vals."""
    AT = np.zeros((S, N, N), dtype=np.float32)
    for s in range(S):
        np.add.at(AT[s], (sup_cols[s].astype(np.int64),
                          sup_rows[s].astype(np.int64)),
                  sup_vals[s].astype(np.float32))
    return AT


def _bt_sq(AT):
    """BT_s = 2 * AT_s @ AT_s (== (2 A^2)^T)."""
    try:
        from scipy import sparse
        out = []
        for s in range(S):
            sp = sparse.csr_matrix(AT[s])
            out.append(np.asarray((sp @ sp).todense(), dtype=np.float32) * 2.0)
        return out
    except ImportError:
        return [2.0 * (AT[s] @ AT[s]) for s in range(S)]


def _prep_shared(sup_rows, sup_cols, sup_vals, weight, biases):
    AT = _dense_at(sup_rows, sup_cols, sup_vals)
    BT = _bt_sq(AT)
    mats = [AT[0], BT[0], AT[1], BT[1]]
    mx = max(float(np.abs(m).max()) for m in mats)
    scale = float(2.0 ** np.floor(np.log2(120.0 / mx)))

    # am[m, rc, h, p, i, q] = (s*AT_m)[(2h+i)*128+p, rc*512+q]
    am = np.empty((NM, NRC, NPAIR, 128, 2, 512), dtype=_f8)
    for m in range(NM):
        q = np.asarray(mats[m] * scale, dtype=_f8)
        am[m] = q.reshape(NPAIR, 2, 128, NRC, 512).transpose(3, 0, 2, 1, 4)

    W = np.asarray(weight, dtype=np.float32).reshape(F, 5, O)
    v0s = np.ascontiguousarray(
        ((W[:, 0] - W[:, 2] - W[:, 4]) * scale).astype(_bf))
    vcat = np.ascontiguousarray(
        np.concatenate([W[:, 1], W[:, 2], W[:, 3], W[:, 4]],
                       axis=1).astype(_bf))
    bias = np.asarray(biases, dtype=np.float32).reshape(128, 1)
    idnb = np.eye(128, dtype=_bf)
    sc = np.full((128, 1), 1.0 / scale, dtype=np.float32)
    return am, vcat, v0s, bias, idnb, sc


def kernel(inputs, state, sup_rows, sup_cols, sup_vals, weight, biases,
           output_size=128, **_ignored):
    inputs = np.asarray(inputs, dtype=np.float32)
    state = np.asarray(state, dtype=np.float32)
    x = np.concatenate(
        [inputs.reshape(B, N, D), state.reshape(B, N, H)], axis=2)  # [B,N,F]

    am, vcat, v0s, bias, idnb, sc = _prep_shared(
        np.asarray(sup_rows), np.asarray(sup_cols), np.asarray(sup_vals),
        weight, biases)

    if "nc" not in _CACHE:
        _CACHE["nc"] = build_nc()
    nc = _CACHE["nc"]

    in_maps = []
    for c in range(NCORES):
        xc = np.ascontiguousarray(
            x[c * BLOC:(c + 1) * BLOC].transpose(1, 0, 2).reshape(
                NBLK, 128, BLOC * F).astype(_bf))
        in_maps.append({
            "x0": xc, "am": am, "vcat": vcat, "v0s": v0s, "bias": bias,
            "idnb": idnb, "sc": sc,
        })

    res = None
    for attempt in range(3):
        try:
            res = bass_utils.run_bass_kernel_spmd(
                nc, in_maps, core_ids=list(range(NCORES)), trace=False)
            break
        except Exception:
            if attempt == 2:
                raise
            import time as _time
            _time.sleep(15 * (attempt + 1))

    # reassemble: out_core[j, o, n] -> out[b, n, o]
    outs = np.stack([np.asarray(res.results[c]["out"]).astype(np.float32)
                     for c in range(NCORES)])
    full = outs.transpose(0, 1, 3, 2).reshape(B, N, O)
    return np.ascontiguousarray(full.reshape(B, N * O))


# revision 3
# speedup vs baseline: 1.9644x; 1.0863x over previous
"""DiffusionGraphConv Trainium2 kernel (8-core SPMD, fp8-DoubleRow design).

Math (per reference, B=32, N=4096, F=128, O=128):
  x = concat(inputs, state)  -> [B, N, F];  x1 = A_s x ; x2 = (2A_s^2 - I) x
  out = sum_m xs_m @ W_m + bias

Reassociation: with B_s = 2 A_s^2 and Y_m = x @ W_m:
  out = x (W0 - W2 - W4) + A_1 Y_1 + B_1 Y_2 + A_2 Y_3 + B_2 Y_4
No Chebyshev dependency chain: all four propagations stream their (dense,
fp8-quantized, power-of-2-scaled) matrix once through the TensorE in
DoubleRow mode (256-row contraction per instruction, 2x bf16 MAC rate on HW).

Sharding: batch across 8 cores (4 batches/core). Host supplies x0 already
transposed (x0T[j] = [F, N] bf16), so the device does no transposes.

Per-core device schedule:
  1. Y-build (k-outer): per (k-tile, j): psum = x0T-tile^T @ [W1..W4],
     evacuated to fp8 Y, alternating ACT/DVE
  2. out-pass: per 512-node chunk rc, per j: one PSUM chain =
     start matmul (s*(W0-W2-W4))^T @ x0T-chunk + 64 fp8 DoubleRow matmuls
     (Y[k-pair] stationary, s*AT_m[k-pair, chunk] moving),
     ACT evacuation out = psum * (1/s) + bias -> bf16 -> DRAM
"""

import numpy as np
import ml_dtypes

import concourse.bass as bass
import concourse.tile as tile
from concourse import bacc, mybir
from concourse import bass_utils

B, N, D, H, O, S = 32, 4096, 64, 64, 128, 2
F = D + H                    # 128
NCORES = 8
BLOC = B // NCORES           # 4 batches per core
NBLK = N // 128              # 32 n-tiles
NRC = N // 512               # 8 output-node chunks
NPAIR = NBLK // 2            # 16 DoubleRow k-pairs
NM = 4                       # propagation matrices: A1, B1, A2, B2

F32 = mybir.dt.float32
BF16 = mybir.dt.bfloat16
FP8 = mybir.dt.float8e4
DRMODE = mybir.MatmulPerfMode.DoubleRow

_f8 = ml_dtypes.float8_e4m3
_bf = ml_dtypes.bfloat16

_CACHE = {}


def build_nc():
    nc = bacc.Bacc("TRN2", target_bir_lowering=False, debug=False)

    # x0t[j, rc] = x0T_j[f, rc*512:(rc+1)*512]  (x0T_j = [F, N])
    x0t_d = nc.dram_tensor("x0t", [BLOC, NRC, 128, 512], BF16,
                           kind="ExternalInput")
    # am[m, rc, h, p, i, q] = (s*AT_m)[(2h+i)*128+p, rc*512+q]
    am_d = nc.dram_tensor("am", [NM, NRC, NPAIR, 128, 2, 512], FP8,
                          kind="ExternalInput")
    vcat_d = nc.dram_tensor("vcat", [128, 512], BF16, kind="ExternalInput")
    v0s_d = nc.dram_tensor("v0s", [128, 128], BF16, kind="ExternalInput")
    bias_d = nc.dram_tensor("bias", [128, 1], F32, kind="ExternalInput")
    sc_d = nc.dram_tensor("sc", [128, 1], F32, kind="ExternalInput")
    # out[j, o, n] = out_core^T per batch
    out_d = nc.dram_tensor("out", [BLOC, 128, N], BF16, kind="ExternalOutput")

    with tile.TileContext(nc) as tc:
        with (
            tc.tile_pool(name="big", bufs=1) as big,
            tc.tile_pool(name="amp", bufs=48) as amp,
            tc.tile_pool(name="stg", bufs=1) as stg,
            tc.tile_pool(name="pst", bufs=3, space=bass.MemorySpace.PSUM) as pst,
            tc.tile_pool(name="pso", bufs=5, space=bass.MemorySpace.PSUM) as pso,
        ):
            # ---- resident loads ----
            x0t = big.tile([128, BLOC, N], BF16, tag="x0t")
            for rc in range(NRC):
                for j in range(BLOC):
                    nc.sync.dma_start(
                        x0t[:, j, rc * 512:(rc + 1) * 512], x0t_d[j, rc])
            vcat = big.tile([128, 512], BF16, tag="vcat")
            nc.sync.dma_start(vcat[:], vcat_d[:])
            v0s = big.tile([128, 128], BF16, tag="v0s")
            nc.sync.dma_start(v0s[:], v0s_d[:])
            bias_sb = big.tile([128, 1], F32, tag="bias")
            nc.sync.dma_start(bias_sb[:], bias_d[:])
            sc_sb = big.tile([128, 1], F32, tag="sc")
            nc.sync.dma_start(sc_sb[:], sc_d[:])
            zr = big.tile([128, 1], F32, tag="zr")
            nc.scalar.memzero(zr[:])

            # Y[j] fp8 [128 n-part, k, 4m*128o]
            y = big.tile([128, BLOC, NBLK, 512], FP8, tag="y")

            # ---- phase 1: Y-build (k-outer so DR chains unblock early) ----
            for k in range(NBLK):
                for j in range(BLOC):
                    py = pst.tile([128, 512], F32, tag="py")
                    nc.tensor.matmul(
                        py[:], x0t[:, j, k * 128:(k + 1) * 128], vcat[:],
                        start=True, stop=True)
                    if (k * BLOC + j) % 2 == 0:
                        nc.scalar.copy(y[:, j, k, :], py[:])
                    else:
                        nc.vector.tensor_scalar_add(
                            y[:, j, k, :], py[:], zr[:, 0:1])

            # ---- phase 2: out-pass ----
            for rc in range(NRC):
                po = [pso.tile([128, 512], F32, tag="po",
                               name=f"po_{rc}_{j}") for j in range(BLOC)]
                for j in range(BLOC):
                    nc.tensor.matmul(
                        po[j][:], v0s[:],
                        x0t[:, j, rc * 512:(rc + 1) * 512],
                        start=True, stop=False)
                for m in range(NM):
                    for h in range(NPAIR):
                        at = amp.tile([128, 2, 512], FP8, tag="am")
                        nc.sync.dma_start(at[:], am_d[m, rc, h])
                        last = (m == NM - 1) and (h == NPAIR - 1)
                        for j in range(BLOC):
                            nc.tensor.matmul(
                                po[j][:],
                                y[:, j, 2 * h:2 * h + 2,
                                  m * 128:(m + 1) * 128],
                                at[:],
                                start=False, stop=last,
                                perf_mode=DRMODE)
                for j in range(BLOC):
                    ot = stg.tile([128, 512], BF16, tag="ot", bufs=8)
                    nc.scalar.activation(
                        ot[:], po[j][:],
                        mybir.ActivationFunctionType.Identity,
                        bias=bias_sb[:, 0:1], scale=sc_sb[:, 0:1])
                    nc.sync.dma_start(
                        out_d[j, :, rc * 512:(rc + 1) * 512], ot[:])

    nc.compile()
    return nc


def _dense_at(sup_rows, sup_cols, sup_vals):
    """AT_s dense [S, N, N]: AT[c, r] = sum vals."""
    AT = np.zeros((S, N, N), dtype=np.float32)
    for s in range(S):
        np.add.at(AT[s], (sup_cols[s].astype(np.int64),
                          sup_rows[s].astype(np.int64)),
                  sup_vals[s].astype(np.float32))
    return AT


def _bt_sq(AT):
    """BT_s = 2 * AT_s @ AT_s (== (2 A^2)^T)."""
    try:
        from scipy import sparse
        out = []
        for s in range(S):
            sp = sparse.csr_matrix(AT[s])
            out.append(np.asarray((sp @ sp).todense(), dtype=np.float32) * 2.0)
        return out
    except ImportError:
        return [2.0 * (AT[s] @ AT[s]) for s in range(S)]


def _prep_shared(sup_rows, sup_cols, sup_vals, weight, biases):
    AT = _dense_at(sup_rows, sup_cols, sup_vals)
    BT = _bt_sq(AT)
    mats = [AT[0], BT[0], AT[1], BT[1]]
    mx = max(float(np.abs(m).max()) for m in mats)
    scale = float(2.0 ** np.floor(np.log2(120.0 / mx)))

    # am[m, rc, h, p, i, q] = (s*AT_m)[(2h+i)*128+p, rc*512+q]
    am = np.empty((NM, NRC, NPAIR, 128, 2, 512), dtype=_f8)
    for m in range(NM):
        q = np.asarray(mats[m] * scale, dtype=_f8)
        am[m] = q.reshape(NPAIR, 2, 128, NRC, 512).transpose(3, 0, 2, 1, 4)

    W = np.asarray(weight, dtype=np.float32).reshape(F, 5, O)
    v0s = np.ascontiguousarray(
        ((W[:, 0] - W[:, 2] - W[:, 4]) * scale).astype(_bf))
    vcat = np.ascontiguousarray(
        np.concatenate([W[:, 1], W[:, 2], W[:, 3], W[:, 4]],
                       axis=1).astype(_bf))
    bias = np.asarray(biases, dtype=np.float32).reshape(128, 1)
    sc = np.full((128, 1), 1.0 / scale, dtype=np.float32)
    return am, vcat, v0s, bias, sc


def kernel(inputs, state, sup_rows, sup_cols, sup_vals, weight, biases,
           output_size=128, **_ignored):
    inputs = np.asarray(inputs, dtype=np.float32)
    state = np.asarray(state, dtype=np.float32)
    x = np.concatenate(
        [inputs.reshape(B, N, D), state.reshape(B, N, H)], axis=2)  # [B,N,F]

    am, vcat, v0s, bias, sc = _prep_shared(
        np.asarray(sup_rows), np.asarray(sup_cols), np.asarray(sup_vals),
        weight, biases)

    if "nc" not in _CACHE:
        _CACHE["nc"] = build_nc()
    nc = _CACHE["nc"]

    in_maps = []
    for c in range(NCORES):
        # x0T per core: [F, BLOC, N] -> chunks [BLOC, NRC, 128, 512]
        xt = x[c * BLOC:(c + 1) * BLOC].transpose(2, 0, 1)  # [F, BLOC, N]
        xtc = np.ascontiguousarray(
            xt.reshape(128, BLOC, NRC, 512).transpose(1, 2, 0, 3).astype(_bf))
        in_maps.append({
            "x0t": xtc, "am": am, "vcat": vcat, "v0s": v0s, "bias": bias,
            "sc": sc,
        })

    res = None
    for attempt in range(3):
        try:
            res = bass_utils.run_bass_kernel_spmd(
                nc, in_maps, core_ids=list(range(NCORES)), trace=False)
            break
        except Exception:
            if attempt == 2:
                raise
            import time as _time
            _time.sleep(15 * (attempt + 1))

    # reassemble: out_core[j, o, n] -> out[b, n, o]
    outs = np.stack([np.asarray(res.results[c]["out"]).astype(np.float32)
                     for c in range(NCORES)])
    full = outs.transpose(0, 1, 3, 2).reshape(B, N, O)
    return np.ascontiguousarray(full.reshape(B, N * O))


# revision 4
# speedup vs baseline: 2.0046x; 1.0205x over previous
"""DiffusionGraphConv Trainium2 kernel (8-core SPMD, fp8-DoubleRow design).

Math (per reference, B=32, N=4096, F=128, O=128):
  x = concat(inputs, state)  -> [B, N, F];  x1 = A_s x ; x2 = (2A_s^2 - I) x
  out = sum_m xs_m @ W_m + bias

Reassociation: with B_s = 2 A_s^2 and Y_m = x @ W_m:
  out = x (W0 - W2 - W4) + A_1 Y_1 + B_1 Y_2 + A_2 Y_3 + B_2 Y_4
No Chebyshev dependency chain: all four propagations stream their (dense,
fp8-quantized, power-of-2-scaled) matrix once through the TensorE in
DoubleRow mode (256-row contraction per instruction, 2x bf16 MAC rate on HW).

Sharding: batch across 8 cores (4 batches/core). Host supplies x0 already
transposed (x0T[j] = [F, N] bf16), so the device does no transposes.

Per-core device schedule:
  1. Y-build (k-outer): per (k-tile, j): psum = x0T-tile^T @ [W1..W4],
     evacuated to fp8 Y, alternating ACT/DVE
  2. out-pass: per 512-node chunk rc, per j: one PSUM chain =
     start matmul (s*(W0-W2-W4))^T @ x0T-chunk + 64 fp8 DoubleRow matmuls
     (Y[k-pair] stationary, s*AT_m[k-pair, chunk] moving),
     ACT evacuation out = psum * (1/s) + bias -> bf16 -> DRAM
"""

import numpy as np
import ml_dtypes

import concourse.bass as bass
import concourse.tile as tile
from concourse import bacc, mybir
from concourse import bass_utils

B, N, D, H, O, S = 32, 4096, 64, 64, 128, 2
F = D + H                    # 128
NCORES = 8
BLOC = B // NCORES           # 4 batches per core
NBLK = N // 128              # 32 n-tiles
NRC = N // 512               # 8 output-node chunks
NPAIR = NBLK // 2            # 16 DoubleRow k-pairs
NM = 4                       # propagation matrices: A1, B1, A2, B2

F32 = mybir.dt.float32
BF16 = mybir.dt.bfloat16
FP8 = mybir.dt.float8e4
DRMODE = mybir.MatmulPerfMode.DoubleRow

_f8 = ml_dtypes.float8_e4m3
_bf = ml_dtypes.bfloat16

_CACHE = {}


def build_nc():
    nc = bacc.Bacc("TRN2", target_bir_lowering=False, debug=False)

    # x0t[j, rc] = x0T_j[f, rc*512:(rc+1)*512]  (x0T_j = [F, N])
    x0t_d = nc.dram_tensor("x0t", [BLOC, NRC, 128, 512], BF16,
                           kind="ExternalInput")
    # am[m, rc, g, p, i, q] = (s*AT_m)[(4g+i)*128+p, rc*512+q]
    am_d = nc.dram_tensor("am", [NM, NRC, NPAIR // 2, 128, 4, 512], FP8,
                          kind="ExternalInput")
    vcat_d = nc.dram_tensor("vcat", [128, 512], BF16, kind="ExternalInput")
    v0s_d = nc.dram_tensor("v0s", [128, 128], BF16, kind="ExternalInput")
    bias_d = nc.dram_tensor("bias", [128, 1], F32, kind="ExternalInput")
    sc_d = nc.dram_tensor("sc", [128, 1], F32, kind="ExternalInput")
    # out[j, o, n] = out_core^T per batch
    out_d = nc.dram_tensor("out", [BLOC, 128, N], BF16, kind="ExternalOutput")

    with tile.TileContext(nc) as tc:
        with (
            tc.tile_pool(name="big", bufs=1) as big,
            tc.tile_pool(name="amp", bufs=24) as amp,
            tc.tile_pool(name="stg", bufs=1) as stg,
            tc.tile_pool(name="pst", bufs=3, space=bass.MemorySpace.PSUM) as pst,
            tc.tile_pool(name="pso", bufs=5, space=bass.MemorySpace.PSUM) as pso,
        ):
            # ---- resident loads (small tensors first: Y-build needs vcat) --
            vcat = big.tile([128, 512], BF16, tag="vcat")
            nc.sync.dma_start(vcat[:], vcat_d[:])
            v0s = big.tile([128, 128], BF16, tag="v0s")
            nc.sync.dma_start(v0s[:], v0s_d[:])
            bias_sb = big.tile([128, 1], F32, tag="bias")
            nc.sync.dma_start(bias_sb[:], bias_d[:])
            sc_sb = big.tile([128, 1], F32, tag="sc")
            nc.sync.dma_start(sc_sb[:], sc_d[:])
            zr = big.tile([128, 1], F32, tag="zr")
            nc.scalar.memzero(zr[:])
            x0t = big.tile([128, BLOC, N], BF16, tag="x0t")
            for rc in range(NRC):
                for j in range(BLOC):
                    nc.sync.dma_start(
                        x0t[:, j, rc * 512:(rc + 1) * 512], x0t_d[j, rc])

            # Y[j] fp8 [128 n-part, k, 4m*128o]
            y = big.tile([128, BLOC, NBLK, 512], FP8, tag="y")

            # ---- phase 1: Y-build (k-outer so DR chains unblock early) ----
            for k in range(NBLK):
                for j in range(BLOC):
                    py = pst.tile([128, 512], F32, tag="py")
                    nc.tensor.matmul(
                        py[:], x0t[:, j, k * 128:(k + 1) * 128], vcat[:],
                        start=True, stop=True)
                    if (k * BLOC + j) % 2 == 0:
                        nc.scalar.copy(y[:, j, k, :], py[:])
                    else:
                        nc.vector.tensor_scalar_add(
                            y[:, j, k, :], py[:], zr[:, 0:1])

            # ---- phase 2: out-pass ----
            for rc in range(NRC):
                po = [pso.tile([128, 512], F32, tag="po",
                               name=f"po_{rc}_{j}") for j in range(BLOC)]
                for j in range(BLOC):
                    nc.tensor.matmul(
                        po[j][:], v0s[:],
                        x0t[:, j, rc * 512:(rc + 1) * 512],
                        start=True, stop=False)
                for m in range(NM):
                    for g in range(NPAIR // 2):
                        at = amp.tile([128, 4, 512], FP8, tag="am")
                        nc.sync.dma_start(at[:], am_d[m, rc, g])
                        for i2 in (0, 2):
                            last = (m == NM - 1) and (g == NPAIR // 2 - 1) \
                                and (i2 == 2)
                            k0 = 4 * g + i2
                            for j in range(BLOC):
                                nc.tensor.matmul(
                                    po[j][:],
                                    y[:, j, k0:k0 + 2,
                                      m * 128:(m + 1) * 128],
                                    at[:, i2:i2 + 2, :],
                                    start=False, stop=last,
                                    perf_mode=DRMODE)
                for j in range(BLOC):
                    ot = stg.tile([128, 512], BF16, tag="ot", bufs=8)
                    nc.scalar.activation(
                        ot[:], po[j][:],
                        mybir.ActivationFunctionType.Identity,
                        bias=bias_sb[:, 0:1], scale=sc_sb[:, 0:1])
                    nc.sync.dma_start(
                        out_d[j, :, rc * 512:(rc + 1) * 512], ot[:])

    nc.compile()
    return nc


def _dense_at(sup_rows, sup_cols, sup_vals):
    """AT_s dense [S, N, N]: AT[c, r] = sum vals."""
    AT = np.zeros((S, N, N), dtype=np.float32)
    for s in range(S):
        np.add.at(AT[s], (sup_cols[s].astype(np.int64),
                          sup_rows[s].astype(np.int64)),
                  sup_vals[s].astype(np.float32))
    return AT


def _bt_sq(AT):
    """BT_s = 2 * AT_s @ AT_s (== (2 A^2)^T)."""
    try:
        from scipy import sparse
        out = []
        for s in range(S):
            sp = sparse.csr_matrix(AT[s])
            out.append(np.asarray((sp @ sp).todense(), dtype=np.float32) * 2.0)
        return out
    except ImportError:
        return [2.0 * (AT[s] @ AT[s]) for s in range(S)]


def _prep_shared(sup_rows, sup_cols, sup_vals, weight, biases):
    AT = _dense_at(sup_rows, sup_cols, sup_vals)
    BT = _bt_sq(AT)
    mats = [AT[0], BT[0], AT[1], BT[1]]
    mx = max(float(np.abs(m).max()) for m in mats)
    scale = float(2.0 ** np.floor(np.log2(120.0 / mx)))

    # am[m, rc, g, p, i, q] = (s*AT_m)[(4g+i)*128+p, rc*512+q]
    am = np.empty((NM, NRC, NPAIR // 2, 128, 4, 512), dtype=_f8)
    for m in range(NM):
        q = np.asarray(mats[m] * scale, dtype=_f8)
        am[m] = q.reshape(NPAIR // 2, 4, 128, NRC, 512).transpose(
            3, 0, 2, 1, 4)

    W = np.asarray(weight, dtype=np.float32).reshape(F, 5, O)
    v0s = np.ascontiguousarray(
        ((W[:, 0] - W[:, 2] - W[:, 4]) * scale).astype(_bf))
    vcat = np.ascontiguousarray(
        np.concatenate([W[:, 1], W[:, 2], W[:, 3], W[:, 4]],
                       axis=1).astype(_bf))
    bias = np.asarray(biases, dtype=np.float32).reshape(128, 1)
    sc = np.full((128, 1), 1.0 / scale, dtype=np.float32)
    return am, vcat, v0s, bias, sc


def kernel(inputs, state, sup_rows, sup_cols, sup_vals, weight, biases,
           output_size=128, **_ignored):
    inputs = np.asarray(inputs, dtype=np.float32)
    state = np.asarray(state, dtype=np.float32)
    x = np.concatenate(
        [inputs.reshape(B, N, D), state.reshape(B, N, H)], axis=2)  # [B,N,F]

    am, vcat, v0s, bias, sc = _prep_shared(
        np.asarray(sup_rows), np.asarray(sup_cols), np.asarray(sup_vals),
        weight, biases)

    if "nc" not in _CACHE:
        _CACHE["nc"] = build_nc()
    nc = _CACHE["nc"]

    in_maps = []
    for c in range(NCORES):
        # x0T per core: [F, BLOC, N] -> chunks [BLOC, NRC, 128, 512]
        xt = x[c * BLOC:(c + 1) * BLOC].transpose(2, 0, 1)  # [F, BLOC, N]
        xtc = np.ascontiguousarray(
            xt.reshape(128, BLOC, NRC, 512).transpose(1, 2, 0, 3).astype(_bf))
        in_maps.append({
            "x0t": xtc, "am": am, "vcat": vcat, "v0s": v0s, "bias": bias,
            "sc": sc,
        })

    res = None
    for attempt in range(3):
        try:
            res = bass_utils.run_bass_kernel_spmd(
                nc, in_maps, core_ids=list(range(NCORES)), trace=False)
            break
        except Exception:
            if attempt == 2:
                raise
            import time as _time
            _time.sleep(15 * (attempt + 1))

    # reassemble: out_core[j, o, n] -> out[b, n, o]
    outs = np.stack([np.asarray(res.results[c]["out"]).astype(np.float32)
                     for c in range(NCORES)])
    full = outs.transpose(0, 1, 3, 2).reshape(B, N, O)
    return np.ascontiguousarray(full.reshape(B, N * O))
